# revision 14
# baseline (speedup 1.0000x reference)
import sys

if '/opt/trn_rl_repo' not in sys.path:
    sys.path.insert(0, '/opt/trn_rl_repo')

import numpy as np

# Model dims (hardcoded from the problem spec)
B, C, N = 4, 512, 2048
NH, D = 8, 64          # heads, head dim
HID = 1024             # mlp hidden
NLOC = N // 2          # sequence half per core
CG = C // 128          # channel groups of 128
MT = N // 128          # key tiles of 128 over full sequence
QT = NLOC // 512       # query tiles of 512 over local half
BN_EPS = 1e-5

SC = 16.0              # fp8 weight prescale
ESH = 16.0             # exp output downshift (exp/16) so attnout fits fp8
LN2 = float(np.log(2.0))
# Schraudolph int8 fp8e4m3 exp: bits = s*C1 + C2 computes exp(s/2048)/16
C1 = 8.0 / (2048.0 * LN2)
C2 = 7 * 8 + 0.5 - 0.36 - 4 * 8   # exponent bias - 4 steps for /16

# exp engine schedule per key tile (Act/DVE only: GPSIMD cannot read PSUM)
PAT_EVEN = ['A', 'D', 'A', 'D', 'A', 'D', 'A', 'D',
            'A', 'D', 'A', 'D', 'A', 'D', 'A', 'D']   # 8A/8D
PAT_ODD = ['A', 'D', 'A', 'D', 'A', 'D', 'A', 'A',
           'A', 'D', 'A', 'D', 'A', 'D', 'A', 'D']    # 9A/7D
RELU_ENG = ['A', 'D', 'A', 'D', 'A', 'D', 'A', 'D']

_CACHE = {}


def _build_nc(repeat=1):
    import concourse.bacc as bacc
    import concourse.bass as bass
    import concourse.tile as tile
    import concourse.mybir as mybir
    from contextlib import ExitStack

    F32 = mybir.dt.float32
    BF16 = mybir.dt.bfloat16
    FP8 = mybir.dt.float8e4
    I8 = mybir.dt.int8
    AF = mybir.ActivationFunctionType
    ALU = mybir.AluOpType
    DR = mybir.MatmulPerfMode.DoubleRow

    nc = bacc.Bacc("TRN2")

    x8_d = nc.dram_tensor("x8", [C, N], FP8, kind="ExternalInput")
    xsb_d = nc.dram_tensor("xsb", [C, NLOC], BF16, kind="ExternalInput")
    wq_d = nc.dram_tensor("wq8", [C, C], FP8, kind="ExternalInput")
    wk_d = nc.dram_tensor("wk8", [C, C], FP8, kind="ExternalInput")
    wv_d = nc.dram_tensor("wv8", [C, C], FP8, kind="ExternalInput")
    wp_d = nc.dram_tensor("wp8", [D, NH * C], FP8, kind="ExternalInput")
    w1_d = nc.dram_tensor("w18", [C, HID], FP8, kind="ExternalInput")
    w2_d = nc.dram_tensor("w28", [HID, C], FP8, kind="ExternalInput")
    bns_d = nc.dram_tensor("bns", [C, 1], F32, kind="ExternalInput")
    bnb_d = nc.dram_tensor("bnb", [C, 1], F32, kind="ExternalInput")
    y_d = nc.dram_tensor("y", [C, NLOC], F32, kind="ExternalOutput")

    def exp_op(eng, out_ap, out_i8, in_ps, ebias):
        if eng == 'A':
            nc.scalar.activation(out_ap, in_ps, AF.Exp,
                                 bias=ebias[:, 0:1], scale=1.0 / 2048.0)
        elif eng == 'D':
            nc.vector.tensor_scalar(out=out_i8, in0=in_ps,
                                    scalar1=C1, scalar2=C2,
                                    op0=ALU.mult, op1=ALU.add)
        else:
            nc.gpsimd.tensor_scalar(out=out_i8, in0=in_ps,
                                    scalar1=C1, scalar2=C2,
                                    op0=ALU.mult, op1=ALU.add)

    def emit_body(tc, pers, pools):
        (x8, xsb, wq_sb, wk_sb, wv_sb, wp_sb, w1_sb, w2_sb, bns_sb, bnb_sb,
         ebias, k_sb, q2_sb, vT, attnout, y1, y18, y1sb, h8) = pers
        mm, oPp, eTp, nrm, nrm_dram, outp = pools

        if True:
            # ---------------- Phase 1: qkv ----------------
            for g in range(CG):
                for mc in range(2):
                    ps = mm.tile([128, 1024], F32, tag="mm")
                    for nn2 in range(2):
                        for i in range(2):
                            nc.tensor.matmul(
                                ps[:, nn2 * 512:(nn2 + 1) * 512],
                                wk_sb[:, 2 * i:2 * i + 2, g * 128:(g + 1) * 128],
                                x8[:, 2 * i:2 * i + 2,
                                   mc * 1024 + nn2 * 512:mc * 1024 + (nn2 + 1) * 512],
                                start=(i == 0), stop=(i == 1), perf_mode=DR)
                    nc.vector.tensor_copy(
                        out=k_sb[:, g, mc * 8:(mc + 1) * 8, 0:128],
                        in_=ps.rearrange("p (a b) -> p a b", b=128))
                ps = mm.tile([128, 1024], F32, tag="mm")
                for nn2 in range(2):
                    for i in range(2):
                        nc.tensor.matmul(
                            ps[:, nn2 * 512:(nn2 + 1) * 512],
                            wq_sb[:, 2 * i:2 * i + 2, g * 128:(g + 1) * 128],
                            x8[:, 2 * i:2 * i + 2, nn2 * 512:(nn2 + 1) * 512],
                            start=(i == 0), stop=(i == 1), perf_mode=DR)
                # even-head rows -> cols 0:512, odd-head rows -> cols 512:1024
                nc.scalar.copy(
                    out=q2_sb[0:64, g, :, 0:512],
                    in_=ps[0:64, :].rearrange("p (a b) -> p a b", b=512))
                nc.scalar.copy(
                    out=q2_sb[64:128, g, :, 512:1024],
                    in_=ps[64:128, :].rearrange("p (a b) -> p a b", b=512))
                for mt in range(4 * g, 4 * g + 4):
                    ps = mm.tile([128, 1024], F32, tag="mm")
                    for i in range(2):
                        nc.tensor.matmul(
                            ps[:, 0:512],
                            x8[:, 2 * i:2 * i + 2, mt * 128:(mt + 1) * 128],
                            wv_sb[:, 2 * i:2 * i + 2, :],
                            start=(i == 0), stop=(i == 1), perf_mode=DR)
                    nc.scalar.copy(
                        out=vT[:, mt, :, 0:64],
                        in_=ps[:, 0:512].rearrange("p (h e) -> p h e", e=64))

            # ---------------- Phase 2: attention units + interleaved mlp ----
            def unit(qc, hp, uidx):
                hA, hB = 2 * hp, 2 * hp + 1
                pat = PAT_EVEN if uidx % 2 == 0 else PAT_ODD
                oP = oPp.tile([68, 1024], F32, tag="o")
                eT = eTp.tile([128, MT, 1024], FP8, tag="e")
                for mt in range(MT):
                    sc = mm.tile([128, 1024], F32, tag="mm")
                    kk = k_sb[:, hp, mt, :].rearrange("p (t m) -> p t m", t=2)
                    qq = q2_sb[:, hp, qc, :].rearrange("p (t n) -> p t n", t=2)
                    nc.tensor.matmul(sc[:, 0:512], kk, qq[:, :, 0:512],
                                     start=True, stop=True, perf_mode=DR)
                    nc.tensor.matmul(sc[:, 512:1024], kk, qq[:, :, 512:1024],
                                     start=True, stop=True, perf_mode=DR)
                    exp_op(pat[mt], eT[:, mt, :], eT[:, mt, :].bitcast(I8),
                           sc, ebias)
                    if mt % 2 == 1:
                        i = mt - 1
                        nc.tensor.matmul(
                            oP[:, 0:512], vT[:, i:i + 2, hA, :],
                            eT[:, i:i + 2, 0:512],
                            start=(i == 0), stop=(i == MT - 2), perf_mode=DR)
                        nc.tensor.matmul(
                            oP[:, 512:1024], vT[:, i:i + 2, hB, :],
                            eT[:, i:i + 2, 512:1024],
                            start=(i == 0), stop=(i == MT - 2), perf_mode=DR)
                r = nrm.tile([1, 1024], F32, tag="r")
                nc.vector.reciprocal(r, oP[64:65, :])
                r_dram = nrm_dram.tile([1, 1024], F32, tag="rd")
                nc.sync.dma_start(out=r_dram, in_=r[0:1, :])
                bc = nrm.tile([64, 1024], F32, tag="bc")
                rsrc = r_dram[0:1, :]
                bsrc = bass.AP(tensor=rsrc.tensor, offset=rsrc.offset,
                               ap=[[0, 64]] + [list(p) for p in rsrc.ap[1:]])
                nc.sync.dma_start(out=bc, in_=bsrc)
                nc.vector.tensor_tensor(
                    out=attnout[:, hA, qc * 512:(qc + 1) * 512],
                    in0=oP[0:64, 0:512], in1=bc[:, 0:512], op=ALU.mult)
                nc.vector.tensor_tensor(
                    out=attnout[:, hB, qc * 512:(qc + 1) * 512],
                    in0=oP[0:64, 512:1024], in1=bc[:, 512:1024], op=ALU.mult)

            def proj_block(nn):
                lo, hi = nn * 512, (nn + 1) * 512
                for g in range(CG):
                    ps = mm.tile([128, 1024], F32, tag="mm")
                    for j in range(NH // 2):
                        nc.tensor.matmul(
                            ps[:, 0:512],
                            wp_sb[:, 2 * j:2 * j + 2, g * 128:(g + 1) * 128],
                            attnout[:, 2 * j:2 * j + 2, lo:hi],
                            start=(j == 0), stop=(j == NH // 2 - 1), perf_mode=DR)
                    tp = nrm.tile([128, 512], F32, tag="tp")
                    nc.scalar.mul(tp, ps[:, 0:512], 1.0 / (SC * SC))
                    nc.gpsimd.tensor_tensor(
                        out=y1[:, g, lo:hi], in0=tp, in1=xsb[:, g, lo:hi],
                        op=ALU.add)
                    nc.gpsimd.tensor_copy(out=y18[:, g, lo:hi],
                                          in_=y1[:, g, lo:hi])
                    nc.gpsimd.tensor_scalar(
                        out=y1sb[:, g, lo:hi], in0=y1[:, g, lo:hi],
                        scalar1=bns_sb[:, g:g + 1], scalar2=bnb_sb[:, g:g + 1],
                        op0=ALU.mult, op1=ALU.add)

            def fc1_block(nn):
                lo, hi = nn * 512, (nn + 1) * 512
                for ho in range(HID // 128):
                    ps = mm.tile([128, 1024], F32, tag="mm")
                    for i in range(2):
                        nc.tensor.matmul(
                            ps[:, 0:512],
                            w1_sb[:, 2 * i:2 * i + 2, ho * 128:(ho + 1) * 128],
                            y18[:, 2 * i:2 * i + 2, lo:hi],
                            start=(i == 0), stop=(i == 1), perf_mode=DR)
                    if RELU_ENG[ho] == 'A':
                        nc.scalar.activation(h8[:, ho, lo:hi], ps[:, 0:512],
                                             AF.Relu, scale=1.0 / SC)
                    else:
                        nc.vector.tensor_scalar(
                            out=h8[:, ho, lo:hi], in0=ps[:, 0:512],
                            scalar1=1.0 / SC, scalar2=0.0,
                            op0=ALU.mult, op1=ALU.max)

            def fc2_block(nn):
                lo, hi = nn * 512, (nn + 1) * 512
                for g in range(CG):
                    ps = mm.tile([128, 1024], F32, tag="mm")
                    for i in range(4):
                        nc.tensor.matmul(
                            ps[:, 0:512],
                            w2_sb[:, 2 * i:2 * i + 2, g * 128:(g + 1) * 128],
                            h8[:, 2 * i:2 * i + 2, lo:hi],
                            start=(i == 0), stop=(i == 3), perf_mode=DR)
                    t2 = nrm.tile([128, 512], F32, tag="t2")
                    nc.scalar.mul(t2, ps[:, 0:512], 1.0 / SC)
                    ob = outp.tile([128, 512], F32, tag="ob")
                    nc.gpsimd.tensor_tensor(
                        out=ob, in0=t2, in1=y1sb[:, g, lo:hi], op=ALU.add)
                    nc.sync.dma_start(out=y_d[g * 128:(g + 1) * 128, lo:hi],
                                      in_=ob)

            for hp in range(4):
                unit(0, hp, hp)
            unit(1, 0, 4)
            proj_block(0)
            unit(1, 1, 5)
            fc1_block(0)
            unit(1, 2, 6)
            fc2_block(0)
            unit(1, 3, 7)
            proj_block(1)
            fc1_block(1)
            fc2_block(1)

    with tile.TileContext(nc) as tc, ExitStack() as ctx:
        pers_p = ctx.enter_context(tc.tile_pool(name="pers", bufs=1))

        x8 = pers_p.tile([128, CG, N], FP8)
        nc.sync.dma_start(out=x8[:, :, :],
                          in_=x8_d.ap().rearrange("(g p) n -> p g n", p=128))
        wk_sb = pers_p.tile([128, CG, C], FP8)
        nc.sync.dma_start(out=wk_sb[:, :, :],
                          in_=wk_d.ap().rearrange("(g p) c -> p g c", p=128))
        wq_sb = pers_p.tile([128, CG, C], FP8)
        nc.sync.dma_start(out=wq_sb[:, :, :],
                          in_=wq_d.ap().rearrange("(g p) c -> p g c", p=128))
        wv_sb = pers_p.tile([128, CG, C], FP8)
        nc.sync.dma_start(out=wv_sb[:, :, :],
                          in_=wv_d.ap().rearrange("(g p) c -> p g c", p=128))
        xsb = pers_p.tile([128, CG, NLOC], BF16)
        nc.sync.dma_start(out=xsb[:, :, :],
                          in_=xsb_d.ap().rearrange("(g p) n -> p g n", p=128))
        wp_sb = pers_p.tile([64, NH, C], FP8)
        nc.sync.dma_start(out=wp_sb.rearrange("p a b -> p (a b)"), in_=wp_d[:, :])
        w1_sb = pers_p.tile([128, CG, HID], FP8)
        nc.sync.dma_start(out=w1_sb[:, :, :],
                          in_=w1_d.ap().rearrange("(g p) h -> p g h", p=128))
        w2_sb = pers_p.tile([128, HID // 128, C], FP8)
        nc.sync.dma_start(out=w2_sb[:, :, :],
                          in_=w2_d.ap().rearrange("(g p) c -> p g c", p=128))
        bns_sb = pers_p.tile([128, CG], F32)
        nc.sync.dma_start(out=bns_sb[:, :],
                          in_=bns_d.ap().rearrange("(g p) one -> p (g one)", p=128))
        bnb_sb = pers_p.tile([128, CG], F32)
        nc.sync.dma_start(out=bnb_sb[:, :],
                          in_=bnb_d.ap().rearrange("(g p) one -> p (g one)", p=128))

        ebias = pers_p.tile([128, 1], F32)
        nc.vector.memset(ebias[:, :], float(-np.log(ESH)))

        # zero-padded score operands (pads written once, data per repeat)
        k_sb = pers_p.tile([128, CG, MT, 256], FP8)
        nc.vector.memset(k_sb[:, :, :, 128:256], 0)
        # q2 cols: [qe 512 | qo 512 | zeros 1024]; qe rows 64:128 and
        # qo rows 0:64 also stay zero
        q2_sb = pers_p.tile([128, CG, QT, 2048], FP8)
        nc.gpsimd.memset(q2_sb[:, :, :, 1024:2048], 0)
        nc.gpsimd.memset(q2_sb[64:128, :, :, 0:512], 0)
        nc.vector.memset(q2_sb[0:64, :, :, 512:1024], 0)
        vT = pers_p.tile([128, MT, NH, 68], FP8)
        nc.vector.memset(vT[:, :, :, 64:65], 1.0)
        nc.vector.memset(vT[:, :, :, 65:68], 0)

        attnout = pers_p.tile([64, NH, NLOC], FP8)
        y1 = pers_p.tile([128, CG, NLOC], F32)
        y18 = pers_p.tile([128, CG, NLOC], FP8)
        y1sb = pers_p.tile([128, CG, NLOC], F32)
        h8 = pers_p.tile([128, HID // 128, NLOC], FP8)

        pers = (x8, xsb, wq_sb, wk_sb, wv_sb, wp_sb, w1_sb, w2_sb, bns_sb,
                bnb_sb, ebias, k_sb, q2_sb, vT, attnout, y1, y18, y1sb, h8)
        mm = ctx.enter_context(tc.tile_pool(name="mm", bufs=2, space="PSUM"))
        oPp = ctx.enter_context(tc.tile_pool(name="oPp", bufs=2, space="PSUM"))
        eTp = ctx.enter_context(tc.tile_pool(name="eTp", bufs=2))
        nrm = ctx.enter_context(tc.tile_pool(name="nrm", bufs=2))
        nrm_dram = ctx.enter_context(tc.tile_pool(name="nrm_dram", bufs=2, space="DRAM"))
        outp = ctx.enter_context(tc.tile_pool(name="outp", bufs=2))
        pools = (mm, oPp, eTp, nrm, nrm_dram, outp)
        for _rep in range(repeat):
            emit_body(tc, pers, pools)

    nc.compile()
    return nc


def _host_prep(x, w_qkv, w_proj, w_fc1, w_fc2, gamma, beta, running_mean, running_var):
    import ml_dtypes
    FP8 = ml_dtypes.float8_e4m3
    x = np.asarray(x, np.float32)
    w_qkv = np.asarray(w_qkv, np.float32)
    s = (np.asarray(gamma, np.float32)
         / np.sqrt(np.asarray(running_var, np.float32) + BN_EPS))
    b = np.asarray(beta, np.float32) - np.asarray(running_mean, np.float32) * s

    wq8 = (w_qkv[0:C].T * SC).astype(FP8)
    wk8 = (w_qkv[C:2 * C].T * SC).astype(FP8)
    wv8 = (w_qkv[2 * C:3 * C].T * SC).astype(FP8)
    wpT = np.asarray(w_proj, np.float32).T * SC * s[None, :]     # [C, C]
    wp8 = np.ascontiguousarray(
        wpT.reshape(NH, D, C).transpose(1, 0, 2)).reshape(D, NH * C).astype(FP8)
    w18 = (np.asarray(w_fc1, np.float32).T * SC).astype(FP8)
    w28 = (np.asarray(w_fc2, np.float32).T * SC * s[None, :]).astype(FP8)

    common = dict(wq8=np.ascontiguousarray(wq8), wk8=np.ascontiguousarray(wk8),
                  wv8=np.ascontiguousarray(wv8), wp8=np.ascontiguousarray(wp8),
                  w18=np.ascontiguousarray(w18), w28=np.ascontiguousarray(w28),
                  bns=s.reshape(C, 1).astype(np.float32),
                  bnb=b.reshape(C, 1).astype(np.float32))
    in_maps = []
    for core in range(8):
        bi, sh = core // 2, core % 2
        xr = np.roll(x[bi], -sh * NLOC, axis=1)
        x8v = np.ascontiguousarray(xr).astype(FP8)
        xsb = np.ascontiguousarray(
            (s[:, None] * xr[:, 0:NLOC] + b[:, None]).astype(ml_dtypes.bfloat16))
        in_maps.append(dict(x8=x8v, xsb=xsb, **common))
    return x, in_maps


def kernel(x, w_qkv, w_proj, w_fc1, w_fc2, gamma, beta,
           running_mean, running_var, **_ignored):
    from concourse.bass_utils import run_bass_kernel_spmd
    if 'nc' not in _CACHE:
        _CACHE['nc'] = _build_nc()
    nc = _CACHE['nc']
    x, in_maps = _host_prep(x, w_qkv, w_proj, w_fc1, w_fc2, gamma, beta,
                            running_mean, running_var)
    res = run_bass_kernel_spmd(nc, in_maps, core_ids=list(range(8)))
    y = np.empty((B, C, N), np.float32)
    for core in range(8):
        bi, sh = core // 2, core % 2
        y[bi][:, sh * NLOC:(sh + 1) * NLOC] = res.results[core]["y"]
    return y


# revision 17
# speedup vs baseline: 1.0153x; 1.0153x over previous
import sys

if '/opt/trn_rl_repo' not in sys.path:
    sys.path.insert(0, '/opt/trn_rl_repo')

import numpy as np

# Model dims (hardcoded from the problem spec)
B, C, N = 4, 512, 2048
NH, D = 8, 64          # heads, head dim
HID = 1024             # mlp hidden
NLOC = N // 2          # sequence half per core
CG = C // 128          # channel groups of 128
MT = N // 128          # m-tiles of 128 over full sequence
BN_EPS = 1e-5

LN2 = float(np.log(2.0))
SCH_C1 = 128.0 / LN2
SCH_C2 = 127 * 128 + 0.5 - 0.045 * 128
# per-(hp,qc) schedule: which key-tiles' exp runs on DVE (int16-bf16 trick)
EXP_DVE = {1, 4, 7, 10, 13}

_CACHE = {}


def _build_nc(repeat=1):
    import concourse.bacc as bacc
    import concourse.bass as bass
    import concourse.tile as tile
    import concourse.mybir as mybir
    from contextlib import ExitStack

    F32R, F32 = mybir.dt.float32r, mybir.dt.float32
    BF16 = mybir.dt.bfloat16
    AF = mybir.ActivationFunctionType
    ALU = mybir.AluOpType

    nc = bacc.Bacc("TRN2")

    x_d = nc.dram_tensor("x", [C, N], F32R, kind="ExternalInput")
    wq_d = nc.dram_tensor("wqT", [C, C], F32R, kind="ExternalInput")
    wk_d = nc.dram_tensor("wkT", [C, C], F32R, kind="ExternalInput")
    wv_d = nc.dram_tensor("wvT", [C, C], F32R, kind="ExternalInput")
    wp_d = nc.dram_tensor("wpT", [C, C], F32R, kind="ExternalInput")
    w1_d = nc.dram_tensor("w1T", [C, HID], F32R, kind="ExternalInput")
    w2_d = nc.dram_tensor("w2T", [HID, C], F32R, kind="ExternalInput")
    bns_d = nc.dram_tensor("bns", [C, 1], F32, kind="ExternalInput")
    bnb_d = nc.dram_tensor("bnb", [C, 1], F32, kind="ExternalInput")
    ones_d = nc.dram_tensor("ones", [128, MT * NH], mybir.dt.bfloat16, kind="ExternalInput")
    y_d = nc.dram_tensor("y", [C, NLOC], F32, kind="ExternalOutput")

    def emit_body(tc, pers_tiles):
        xb, wp_sb, bns_sb, bnb_sb, attnout, y1 = pers_tiles

        with tc.tile_pool(name="attn_data", bufs=1) as ad:
            xb16 = ad.tile([128, CG, N], BF16)
            for g in range(CG):
                nc.gpsimd.dma_start(out=xb16[:, g, :], in_=x_d[g * 128:(g + 1) * 128, :])
            k_sb = ad.tile([128, CG, N], BF16)
            q_sb = ad.tile([128, CG, NLOC], BF16)
            vT = ad.tile([128, MT, NH * 65], BF16)
            # ones columns of vT (col 64 of each 65-wide head block)
            vT_ones = vT.rearrange("p m (h e) -> p (m h) e", e=65)[:, :, 64:65]
            nc.sync.dma_start(out=vT_ones,
                              in_=ones_d[:, :].rearrange("p (a b) -> p a b", b=1))

            # ---------------- Phase 1: qkv ----------------
            with tc.tile_pool(name="qkvw", bufs=1) as qw, \
                 tc.tile_pool(name="ps1", bufs=6, space="PSUM") as ps1:
                wq_sb = qw.tile([128, CG, C], BF16)
                wk_sb = qw.tile([128, CG, C], BF16)
                wv_sb = qw.tile([128, CG, C], BF16)
                for c in range(CG):
                    nc.gpsimd.dma_start(out=wq_sb[:, c, :], in_=wq_d[c * 128:(c + 1) * 128, :])
                    nc.gpsimd.dma_start(out=wk_sb[:, c, :], in_=wk_d[c * 128:(c + 1) * 128, :])
                    nc.gpsimd.dma_start(out=wv_sb[:, c, :], in_=wv_d[c * 128:(c + 1) * 128, :])

                # k over the full sequence
                for g in range(CG):
                    for mc in range(N // 512):
                        ps = ps1.tile([128, 512], F32, tag="ps1")
                        for cc in range(CG):
                            nc.tensor.matmul(
                                ps,
                                wk_sb[:, cc, g * 128:(g + 1) * 128],
                                xb16[:, cc, mc * 512:(mc + 1) * 512],
                                start=(cc == 0), stop=(cc == CG - 1))
                        if mc % 2 == 0:
                            nc.scalar.copy(out=k_sb[:, g, mc * 512:(mc + 1) * 512], in_=ps)
                        else:
                            nc.vector.tensor_copy(out=k_sb[:, g, mc * 512:(mc + 1) * 512], in_=ps)
                # q over local half
                for g in range(CG):
                    for qc in range(NLOC // 512):
                        ps = ps1.tile([128, 512], F32, tag="ps1")
                        for cc in range(CG):
                            nc.tensor.matmul(
                                ps,
                                wq_sb[:, cc, g * 128:(g + 1) * 128],
                                xb16[:, cc, qc * 512:(qc + 1) * 512],
                                start=(cc == 0), stop=(cc == CG - 1))
                        if qc % 2 == 0:
                            nc.scalar.copy(out=q_sb[:, g, qc * 512:(qc + 1) * 512], in_=ps)
                        else:
                            nc.vector.tensor_copy(out=q_sb[:, g, qc * 512:(qc + 1) * 512], in_=ps)
                # vT over full sequence: per m-tile, all heads side by side
                for mt in range(MT):
                    ps = ps1.tile([128, 512], F32, tag="ps1")
                    for cc in range(CG):
                        nc.tensor.matmul(
                            ps,
                            xb16[:, cc, mt * 128:(mt + 1) * 128],
                            wv_sb[:, cc, :],
                            start=(cc == 0), stop=(cc == CG - 1))
                    nc.vector.tensor_copy(
                        out=vT[:, mt, :].rearrange("p (h e) -> p h e", e=65)[:, :, 0:64],
                        in_=ps.rearrange("p (h e) -> p h e", e=64))

            # ---------------- Phase 2: attention ----------------
            with tc.tile_pool(name="eTp", bufs=4) as eTp, \
                 tc.tile_pool(name="nrm", bufs=2) as nrm, \
                 tc.tile_pool(name="nrm_dram", bufs=2, space="DRAM") as nrm_dram, \
                 tc.tile_pool(name="ps_sc", bufs=2, space="PSUM") as ps_sc, \
                 tc.tile_pool(name="ps_o", bufs=4, space="PSUM") as ps_o:
                for hp in range(NH // 2):
                    hA, hB = 2 * hp, 2 * hp + 1
                    for qc in range(NLOC // 512):
                        oA = ps_o.tile([65, 512], F32, tag="po")
                        oB = ps_o.tile([65, 512], F32, tag="po")
                        for mt in range(MT):
                            sc = ps_sc.tile([128, 1024], F32, tag="sc")
                            nc.tensor.matmul(
                                sc[:, 0:512],
                                k_sb[0:64, hp, mt * 128:(mt + 1) * 128],
                                q_sb[0:64, hp, qc * 512:(qc + 1) * 512],
                                start=True, stop=True, tile_position=(0, 0))
                            nc.tensor.matmul(
                                sc[:, 512:1024],
                                k_sb[64:128, hp, mt * 128:(mt + 1) * 128],
                                q_sb[64:128, hp, qc * 512:(qc + 1) * 512],
                                start=True, stop=True, tile_position=(64, 0))
                            eT = eTp.tile([128, 1024], BF16, tag="eT")
                            if mt in EXP_DVE:
                                nc.vector.tensor_scalar(
                                    out=eT.bitcast(mybir.dt.int16), in0=sc,
                                    scalar1=SCH_C1, scalar2=SCH_C2,
                                    op0=ALU.mult, op1=ALU.add)
                            else:
                                nc.scalar.activation(eT, sc, AF.Exp)
                            nc.tensor.matmul(
                                oA, vT[:, mt, hA * 65:(hA + 1) * 65], eT[:, 0:512],
                                start=(mt == 0), stop=(mt == MT - 1))
                            nc.tensor.matmul(
                                oB, vT[:, mt, hB * 65:(hB + 1) * 65], eT[:, 512:1024],
                                start=(mt == 0), stop=(mt == MT - 1))
                        # normalize by the ones-row sums and place into attnout
                        r = nrm.tile([1, 1024], F32, tag="r")
                        nc.vector.reciprocal(r[:, 0:512], oA[64:65, :])
                        nc.vector.reciprocal(r[:, 512:1024], oB[64:65, :])
                        r_dram = nrm_dram.tile([1, 1024], F32, tag="rd")
                        nc.sync.dma_start(out=r_dram, in_=r[0:1, :])
                        bc = nrm.tile([64, 1024], F32, tag="bc")
                        rsrc = r_dram[0:1, :]
                        bsrc = bass.AP(tensor=rsrc.tensor, offset=rsrc.offset,
                                       ap=[[0, 64]] + [list(p) for p in rsrc.ap[1:]])
                        nc.sync.dma_start(out=bc, in_=bsrc)
                        nc.vector.tensor_tensor(
                            out=attnout[0:64, hp, qc * 512:(qc + 1) * 512],
                            in0=oA[0:64, :], in1=bc[:, 0:512], op=ALU.mult)
                        tmpB = nrm.tile([64, 512], BF16, tag="tb")
                        nc.vector.tensor_tensor(
                            out=tmpB, in0=oB[0:64, :], in1=bc[:, 512:1024], op=ALU.mult)
                        nc.sync.dma_start(
                            out=attnout[64:128, hp, qc * 512:(qc + 1) * 512], in_=tmpB)

        # ---------------- Phase 3-5: proj + BN1, MLP, BN2 ----------------
        with tc.tile_pool(name="mlpw", bufs=1) as mw, \
             tc.tile_pool(name="outp", bufs=2) as outp, \
             tc.tile_pool(name="ps_mm", bufs=4, space="PSUM") as ps_mm:
            w1_sb = mw.tile([128, CG, HID], BF16)
            for c in range(CG):
                nc.gpsimd.dma_start(out=w1_sb[:, c, :], in_=w1_d[c * 128:(c + 1) * 128, :])
            w2_sb = mw.tile([128, HID // 128, C], BF16)
            for c in range(HID // 128):
                nc.gpsimd.dma_start(out=w2_sb[:, c, :], in_=w2_d[c * 128:(c + 1) * 128, :])
            h_sb = mw.tile([128, HID // 128, NLOC], BF16)
            y116 = mw.tile([128, CG, NLOC], BF16)

            # proj + BN1 (+ residual x)
            for g in range(CG):
                ps = ps_mm.tile([128, NLOC], F32, tag="mm")
                for cc in range(CG):
                    for qc in range(NLOC // 512):
                        nc.tensor.matmul(
                            ps[:, qc * 512:(qc + 1) * 512],
                            wp_sb[:, cc, g * 128:(g + 1) * 128],
                            attnout[:, cc, qc * 512:(qc + 1) * 512],
                            start=(cc == 0), stop=(cc == CG - 1))
                nc.vector.tensor_tensor(out=y1[:, g, :], in0=ps, in1=xb[:, g, 0:NLOC],
                                        op=ALU.add)
                nc.vector.tensor_scalar(out=y1[:, g, :], in0=y1[:, g, :],
                                        scalar1=bns_sb[:, g:g + 1],
                                        scalar2=bnb_sb[:, g:g + 1],
                                        op0=ALU.mult, op1=ALU.add)
                nc.vector.tensor_copy(out=y116[:, g, :], in_=y1[:, g, :])
            # fc1 + relu
            for go in range(HID // 128):
                ps = ps_mm.tile([128, NLOC], F32, tag="mm")
                for cc in range(CG):
                    for qc in range(NLOC // 512):
                        nc.tensor.matmul(
                            ps[:, qc * 512:(qc + 1) * 512],
                            w1_sb[:, cc, go * 128:(go + 1) * 128],
                            y116[:, cc, qc * 512:(qc + 1) * 512],
                            start=(cc == 0), stop=(cc == CG - 1))
                nc.scalar.activation(h_sb[:, go, :], ps, AF.Relu)
            # fc2 + BN2 (+ residual y1)
            for g in range(CG):
                ps = ps_mm.tile([128, NLOC], F32, tag="mm")
                for hc in range(HID // 128):
                    for qc in range(NLOC // 512):
                        nc.tensor.matmul(
                            ps[:, qc * 512:(qc + 1) * 512],
                            w2_sb[:, hc, g * 128:(g + 1) * 128],
                            h_sb[:, hc, qc * 512:(qc + 1) * 512],
                            start=(hc == 0), stop=(hc == HID // 128 - 1))
                ob = outp.tile([128, NLOC], F32, tag="ob")
                nc.vector.tensor_tensor(out=ob, in0=ps, in1=y1[:, g, :], op=ALU.add)
                nc.vector.tensor_scalar(out=ob, in0=ob,
                                        scalar1=bns_sb[:, g:g + 1],
                                        scalar2=bnb_sb[:, g:g + 1],
                                        op0=ALU.mult, op1=ALU.add)
                nc.sync.dma_start(out=y_d[g * 128:(g + 1) * 128, :], in_=ob)

    with tile.TileContext(nc) as tc, ExitStack() as ctx:
        pers = ctx.enter_context(tc.tile_pool(name="pers", bufs=1))

        xb = pers.tile([128, CG, N], F32R)
        for g in range(CG):
            nc.sync.dma_start(out=xb[:, g, :], in_=x_d[g * 128:(g + 1) * 128, :])
        wp_sb = pers.tile([128, CG, C], BF16)
        for c in range(CG):
            nc.gpsimd.dma_start(out=wp_sb[:, c, :], in_=wp_d[c * 128:(c + 1) * 128, :])
        bns_sb = pers.tile([128, CG], F32)
        bnb_sb = pers.tile([128, CG], F32)
        for g in range(CG):
            nc.sync.dma_start(out=bns_sb[:, g:g + 1], in_=bns_d[g * 128:(g + 1) * 128, :])
            nc.sync.dma_start(out=bnb_sb[:, g:g + 1], in_=bnb_d[g * 128:(g + 1) * 128, :])
        attnout = pers.tile([128, CG, NLOC], BF16)
        y1 = pers.tile([128, CG, NLOC], F32R)

        for _rep in range(repeat):
            emit_body(tc, (xb, wp_sb, bns_sb, bnb_sb, attnout, y1))

    nc.compile()
    return nc


def _host_prep(x, w_qkv, w_proj, w_fc1, w_fc2, gamma, beta, running_mean, running_var):
    x = np.asarray(x, np.float32)
    w_qkv = np.asarray(w_qkv, np.float32)
    bns = (np.asarray(gamma, np.float32)
           / np.sqrt(np.asarray(running_var, np.float32) + BN_EPS))
    bnb = np.asarray(beta, np.float32) - np.asarray(running_mean, np.float32) * bns
    wqT = np.ascontiguousarray(w_qkv[0:C].T) / np.float32(D ** 0.5)
    wkT = np.ascontiguousarray(w_qkv[C:2 * C].T)
    wvT = np.ascontiguousarray(w_qkv[2 * C:3 * C].T)
    wpT = np.ascontiguousarray(np.asarray(w_proj, np.float32).T)
    w1T = np.ascontiguousarray(np.asarray(w_fc1, np.float32).T)
    w2T = np.ascontiguousarray(np.asarray(w_fc2, np.float32).T)
    import ml_dtypes
    ones = np.ones((128, MT * NH), ml_dtypes.bfloat16)
    common = dict(wqT=wqT, wkT=wkT, wvT=wvT, wpT=wpT, w1T=w1T, w2T=w2T,
                  bns=bns.reshape(C, 1).astype(np.float32),
                  bnb=bnb.reshape(C, 1).astype(np.float32), ones=ones)
    in_maps = []
    for core in range(8):
        b, s = core // 2, core % 2
        xb = np.ascontiguousarray(np.roll(x[b], -s * NLOC, axis=1))
        in_maps.append(dict(x=xb, **common))
    return x, in_maps


def kernel(x, w_qkv, w_proj, w_fc1, w_fc2, gamma, beta,
           running_mean, running_var, **_ignored):
    from concourse.bass_utils import run_bass_kernel_spmd
    if 'nc' not in _CACHE:
        _CACHE['nc'] = _build_nc()
    nc = _CACHE['nc']
    x, in_maps = _host_prep(x, w_qkv, w_proj, w_fc1, w_fc2, gamma, beta,
                            running_mean, running_var)
    res = run_bass_kernel_spmd(nc, in_maps, core_ids=list(range(8)))
    y = np.empty((B, C, N), np.float32)
    for core in range(8):
        b, s = core // 2, core % 2
        y[b][:, s * NLOC:(s + 1) * NLOC] = res.results[core]["y"]
    return y



# revision 18
# speedup vs baseline: 2.0612x; 2.0301x over previous
import sys

if '/opt/trn_rl_repo' not in sys.path:
    sys.path.insert(0, '/opt/trn_rl_repo')

import numpy as np

# Model dims (hardcoded from the problem spec)
B, C, N = 4, 512, 2048
NH, D = 8, 64          # heads, head dim
HID = 1024             # mlp hidden
NLOC = N // 2          # sequence half per core
CG = C // 128          # channel groups of 128
MT = N // 128          # m-tiles of 128 over full sequence
BN_EPS = 1e-5

_CACHE = {}


def _build_nc(repeat=1):
    import concourse.bacc as bacc
    import concourse.bass as bass
    import concourse.tile as tile
    import concourse.mybir as mybir
    from contextlib import ExitStack

    F32R, F32 = mybir.dt.float32r, mybir.dt.float32
    BF16 = mybir.dt.bfloat16
    AF = mybir.ActivationFunctionType
    ALU = mybir.AluOpType

    nc = bacc.Bacc("TRN2")

    x_d = nc.dram_tensor("x", [C, N], F32R, kind="ExternalInput")
    wq_d = nc.dram_tensor("wqT", [C, C], F32R, kind="ExternalInput")
    wk_d = nc.dram_tensor("wkT", [C, C], F32R, kind="ExternalInput")
    wv_d = nc.dram_tensor("wvT", [C, C], F32R, kind="ExternalInput")
    wp_d = nc.dram_tensor("wpT", [C, C], F32R, kind="ExternalInput")
    w1_d = nc.dram_tensor("w1T", [C, HID], F32R, kind="ExternalInput")
    w2_d = nc.dram_tensor("w2T", [HID, C], F32R, kind="ExternalInput")
    bns_d = nc.dram_tensor("bns", [C, 1], F32, kind="ExternalInput")
    bnb_d = nc.dram_tensor("bnb", [C, 1], F32, kind="ExternalInput")
    ones_d = nc.dram_tensor("ones", [128, MT * NH], mybir.dt.bfloat16, kind="ExternalInput")
    y_d = nc.dram_tensor("y", [C, NLOC], F32, kind="ExternalOutput")

    def emit_body(tc, pers_tiles):
        xb, wp_sb, bns_sb, bnb_sb, attnout, y1 = pers_tiles

        with tc.tile_pool(name="attn_data", bufs=1) as ad:
            xb16 = ad.tile([128, CG, N], BF16)
            for g in range(CG):
                nc.gpsimd.dma_start(out=xb16[:, g, :], in_=x_d[g * 128:(g + 1) * 128, :])
            k_sb = ad.tile([128, CG, N], BF16)
            q_sb = ad.tile([128, CG, NLOC], BF16)
            vT = ad.tile([128, MT, NH * 65], BF16)
            # ones columns of vT (col 64 of each 65-wide head block)
            vT_ones = vT.rearrange("p m (h e) -> p (m h) e", e=65)[:, :, 64:65]
            nc.sync.dma_start(out=vT_ones,
                              in_=ones_d[:, :].rearrange("p (a b) -> p a b", b=1))

            # ---------------- Phase 1: qkv ----------------
            with tc.tile_pool(name="qkvw", bufs=1) as qw, \
                 tc.tile_pool(name="ps1", bufs=6, space="PSUM") as ps1:
                wq_sb = qw.tile([128, CG, C], BF16)
                wk_sb = qw.tile([128, CG, C], BF16)
                wv_sb = qw.tile([128, CG, C], BF16)
                for c in range(CG):
                    nc.gpsimd.dma_start(out=wq_sb[:, c, :], in_=wq_d[c * 128:(c + 1) * 128, :])
                    nc.gpsimd.dma_start(out=wk_sb[:, c, :], in_=wk_d[c * 128:(c + 1) * 128, :])
                    nc.gpsimd.dma_start(out=wv_sb[:, c, :], in_=wv_d[c * 128:(c + 1) * 128, :])

                # k over the full sequence
                for g in range(CG):
                    for mc in range(N // 512):
                        ps = ps1.tile([128, 512], F32, tag="ps1")
                        for cc in range(CG):
                            nc.tensor.matmul(
                                ps,
                                wk_sb[:, cc, g * 128:(g + 1) * 128],
                                xb16[:, cc, mc * 512:(mc + 1) * 512],
                                start=(cc == 0), stop=(cc == CG - 1))
                        if mc % 2 == 0:
                            nc.scalar.copy(out=k_sb[:, g, mc * 512:(mc + 1) * 512], in_=ps)
                        else:
                            nc.vector.tensor_copy(out=k_sb[:, g, mc * 512:(mc + 1) * 512], in_=ps)
                # q over local half
                for g in range(CG):
                    for qc in range(NLOC // 512):
                        ps = ps1.tile([128, 512], F32, tag="ps1")
                        for cc in range(CG):
                            nc.tensor.matmul(
                                ps,
                                wq_sb[:, cc, g * 128:(g + 1) * 128],
                                xb16[:, cc, qc * 512:(qc + 1) * 512],
                                start=(cc == 0), stop=(cc == CG - 1))
                        if qc % 2 == 0:
                            nc.scalar.copy(out=q_sb[:, g, qc * 512:(qc + 1) * 512], in_=ps)
                        else:
                            nc.vector.tensor_copy(out=q_sb[:, g, qc * 512:(qc + 1) * 512], in_=ps)
                # vT over full sequence: per m-tile, all heads side by side
                for mt in range(MT):
                    ps = ps1.tile([128, 512], F32, tag="ps1")
                    for cc in range(CG):
                        nc.tensor.matmul(
                            ps,
                            xb16[:, cc, mt * 128:(mt + 1) * 128],
                            wv_sb[:, cc, :],
                            start=(cc == 0), stop=(cc == CG - 1))
                    nc.vector.tensor_copy(
                        out=vT[:, mt, :].rearrange("p (h e) -> p h e", e=65)[:, :, 0:64],
                        in_=ps.rearrange("p (h e) -> p h e", e=64))

            # ---------------- Phase 2: attention ----------------
            with tc.tile_pool(name="eTp", bufs=4) as eTp, \
                 tc.tile_pool(name="nrm", bufs=2) as nrm, \
                 tc.tile_pool(name="nrm_dram", bufs=2, space="DRAM") as nrm_dram, \
                 tc.tile_pool(name="ps_sc", bufs=2, space="PSUM") as ps_sc, \
                 tc.tile_pool(name="ps_o", bufs=4, space="PSUM") as ps_o:
                for hp in range(NH // 2):
                    hA, hB = 2 * hp, 2 * hp + 1
                    for qc in range(NLOC // 512):
                        oA = ps_o.tile([65, 512], F32, tag="po")
                        oB = ps_o.tile([65, 512], F32, tag="po")
                        for mt in range(MT):
                            sc = ps_sc.tile([128, 1024], F32, tag="sc")
                            nc.tensor.matmul(
                                sc[:, 0:512],
                                k_sb[0:64, hp, mt * 128:(mt + 1) * 128],
                                q_sb[0:64, hp, qc * 512:(qc + 1) * 512],
                                start=True, stop=True, tile_position=(0, 0))
                            nc.tensor.matmul(
                                sc[:, 512:1024],
                                k_sb[64:128, hp, mt * 128:(mt + 1) * 128],
                                q_sb[64:128, hp, qc * 512:(qc + 1) * 512],
                                start=True, stop=True, tile_position=(64, 0))
                            eT = eTp.tile([128, 1024], BF16, tag="eT")
                            nc.scalar.activation(eT, sc, AF.Exp)
                            nc.tensor.matmul(
                                oA, vT[:, mt, hA * 65:(hA + 1) * 65], eT[:, 0:512],
                                start=(mt == 0), stop=(mt == MT - 1))
                            nc.tensor.matmul(
                                oB, vT[:, mt, hB * 65:(hB + 1) * 65], eT[:, 512:1024],
                                start=(mt == 0), stop=(mt == MT - 1))
                        # normalize by the ones-row sums and place into attnout
                        r = nrm.tile([1, 1024], F32, tag="r")
                        nc.vector.reciprocal(r[:, 0:512], oA[64:65, :])
                        nc.vector.reciprocal(r[:, 512:1024], oB[64:65, :])
                        r_dram = nrm_dram.tile([1, 1024], F32, tag="rd")
                        nc.sync.dma_start(out=r_dram, in_=r[0:1, :])
                        bc = nrm.tile([64, 1024], F32, tag="bc")
                        rsrc = r_dram[0:1, :]
                        bsrc = bass.AP(tensor=rsrc.tensor, offset=rsrc.offset,
                                       ap=[[0, 64]] + [list(p) for p in rsrc.ap[1:]])
                        nc.sync.dma_start(out=bc, in_=bsrc)
                        nc.vector.tensor_tensor(
                            out=attnout[0:64, hp, qc * 512:(qc + 1) * 512],
                            in0=oA[0:64, :], in1=bc[:, 0:512], op=ALU.mult)
                        tmpB = nrm.tile([64, 512], BF16, tag="tb")
                        nc.vector.tensor_tensor(
                            out=tmpB, in0=oB[0:64, :], in1=bc[:, 512:1024], op=ALU.mult)
                        nc.sync.dma_start(
                            out=attnout[64:128, hp, qc * 512:(qc + 1) * 512], in_=tmpB)

        # ---------------- Phase 3-5: proj + BN1, MLP, BN2 ----------------
        with tc.tile_pool(name="mlpw", bufs=1) as mw, \
             tc.tile_pool(name="outp", bufs=2) as outp, \
             tc.tile_pool(name="ps_mm", bufs=4, space="PSUM") as ps_mm:
            w1_sb = mw.tile([128, CG, HID], BF16)
            for c in range(CG):
                nc.gpsimd.dma_start(out=w1_sb[:, c, :], in_=w1_d[c * 128:(c + 1) * 128, :])
            w2_sb = mw.tile([128, HID // 128, C], BF16)
            for c in range(HID // 128):
                nc.gpsimd.dma_start(out=w2_sb[:, c, :], in_=w2_d[c * 128:(c + 1) * 128, :])
            h_sb = mw.tile([128, HID // 128, NLOC], BF16)
            y116 = mw.tile([128, CG, NLOC], BF16)

            # proj + BN1 (+ residual x)
            for g in range(CG):
                ps = ps_mm.tile([128, NLOC], F32, tag="mm")
                for cc in range(CG):
                    for qc in range(NLOC // 512):
                        nc.tensor.matmul(
                            ps[:, qc * 512:(qc + 1) * 512],
                            wp_sb[:, cc, g * 128:(g + 1) * 128],
                            attnout[:, cc, qc * 512:(qc + 1) * 512],
                            start=(cc == 0), stop=(cc == CG - 1))
                nc.vector.tensor_tensor(out=y1[:, g, :], in0=ps, in1=xb[:, g, 0:NLOC],
                                        op=ALU.add)
                nc.vector.tensor_scalar(out=y1[:, g, :], in0=y1[:, g, :],
                                        scalar1=bns_sb[:, g:g + 1],
                                        scalar2=bnb_sb[:, g:g + 1],
                                        op0=ALU.mult, op1=ALU.add)
                nc.vector.tensor_copy(out=y116[:, g, :], in_=y1[:, g, :])
            # fc1 + relu
            for go in range(HID // 128):
                ps = ps_mm.tile([128, NLOC], F32, tag="mm")
                for cc in range(CG):
                    for qc in range(NLOC // 512):
                        nc.tensor.matmul(
                            ps[:, qc * 512:(qc + 1) * 512],
                            w1_sb[:, cc, go * 128:(go + 1) * 128],
                            y116[:, cc, qc * 512:(qc + 1) * 512],
                            start=(cc == 0), stop=(cc == CG - 1))
                nc.scalar.activation(h_sb[:, go, :], ps, AF.Relu)
            # fc2 + BN2 (+ residual y1)
            for g in range(CG):
                ps = ps_mm.tile([128, NLOC], F32, tag="mm")
                for hc in range(HID // 128):
                    for qc in range(NLOC // 512):
                        nc.tensor.matmul(
                            ps[:, qc * 512:(qc + 1) * 512],
                            w2_sb[:, hc, g * 128:(g + 1) * 128],
                            h_sb[:, hc, qc * 512:(qc + 1) * 512],
                            start=(hc == 0), stop=(hc == HID // 128 - 1))
                ob = outp.tile([128, NLOC], F32, tag="ob")
                nc.vector.tensor_tensor(out=ob, in0=ps, in1=y1[:, g, :], op=ALU.add)
                nc.vector.tensor_scalar(out=ob, in0=ob,
                                        scalar1=bns_sb[:, g:g + 1],
                                        scalar2=bnb_sb[:, g:g + 1],
                                        op0=ALU.mult, op1=ALU.add)
                nc.sync.dma_start(out=y_d[g * 128:(g + 1) * 128, :], in_=ob)

    with tile.TileContext(nc) as tc, ExitStack() as ctx:
        pers = ctx.enter_context(tc.tile_pool(name="pers", bufs=1))

        xb = pers.tile([128, CG, N], F32R)
        for g in range(CG):
            nc.sync.dma_start(out=xb[:, g, :], in_=x_d[g * 128:(g + 1) * 128, :])
        wp_sb = pers.tile([128, CG, C], BF16)
        for c in range(CG):
            nc.gpsimd.dma_start(out=wp_sb[:, c, :], in_=wp_d[c * 128:(c + 1) * 128, :])
        bns_sb = pers.tile([128, CG], F32)
        bnb_sb = pers.tile([128, CG], F32)
        for g in range(CG):
            nc.sync.dma_start(out=bns_sb[:, g:g + 1], in_=bns_d[g * 128:(g + 1) * 128, :])
            nc.sync.dma_start(out=bnb_sb[:, g:g + 1], in_=bnb_d[g * 128:(g + 1) * 128, :])
        attnout = pers.tile([128, CG, NLOC], BF16)
        y1 = pers.tile([128, CG, NLOC], F32R)

        for _rep in range(repeat):
            emit_body(tc, (xb, wp_sb, bns_sb, bnb_sb, attnout, y1))

    nc.compile()
    return nc


def _host_prep(x, w_qkv, w_proj, w_fc1, w_fc2, gamma, beta, running_mean, running_var):
    x = np.asarray(x, np.float32)
    w_qkv = np.asarray(w_qkv, np.float32)
    bns = (np.asarray(gamma, np.float32)
           / np.sqrt(np.asarray(running_var, np.float32) + BN_EPS))
    bnb = np.asarray(beta, np.float32) - np.asarray(running_mean, np.float32) * bns
    wqT = np.ascontiguousarray(w_qkv[0:C].T) / np.float32(D ** 0.5)
    wkT = np.ascontiguousarray(w_qkv[C:2 * C].T)
    wvT = np.ascontiguousarray(w_qkv[2 * C:3 * C].T)
    wpT = np.ascontiguousarray(np.asarray(w_proj, np.float32).T)
    w1T = np.ascontiguousarray(np.asarray(w_fc1, np.float32).T)
    w2T = np.ascontiguousarray(np.asarray(w_fc2, np.float32).T)
    import ml_dtypes
    ones = np.ones((128, MT * NH), ml_dtypes.bfloat16)
    common = dict(wqT=wqT, wkT=wkT, wvT=wvT, wpT=wpT, w1T=w1T, w2T=w2T,
                  bns=bns.reshape(C, 1).astype(np.float32),
                  bnb=bnb.reshape(C, 1).astype(np.float32), ones=ones)
    in_maps = []
    for core in range(8):
        b, s = core // 2, core % 2
        xb = np.ascontiguousarray(np.roll(x[b], -s * NLOC, axis=1))
        in_maps.append(dict(x=xb, **common))
    return x, in_maps


def kernel(x, w_qkv, w_proj, w_fc1, w_fc2, gamma, beta,
           running_mean, running_var, **_ignored):
    from concourse.bass_utils import run_bass_kernel_spmd
    if 'nc' not in _CACHE:
        _CACHE['nc'] = _build_nc()
    nc = _CACHE['nc']
    x, in_maps = _host_prep(x, w_qkv, w_proj, w_fc1, w_fc2, gamma, beta,
                            running_mean, running_var)
    res = run_bass_kernel_spmd(nc, in_maps, core_ids=list(range(8)))
    y = np.empty((B, C, N), np.float32)
    for core in range(8):
        b, s = core // 2, core % 2
        y[b][:, s * NLOC:(s + 1) * NLOC] = res.results[core]["y"]
    return y

